# revision 17
# baseline (speedup 1.0000x reference)
"""Trainium2 Bass kernel for a transformer encoder layer.

B=4, S=2048, D=1024, H=16 heads (HD=64), PF=4096, fp32 I/O.

Sharding: 8 cores, core c handles batch c//2, query seq-half c%2 (1024
tokens). Each core computes K/V over its batch's full 2048-token sequence
(duplicated within the pair; ~12% extra flops) so no collectives are needed.

All matmuls run in bf16 (weights cast host-side; activations cast at PSUM
eviction), accumulating in fp32 PSUM. Residual/LayerNorm arithmetic stays
fp32. Attention is kept transposed so no P-matrix transpose is needed:
  srcT = src^T; QT/KT = W^T srcT; V = srcT^T Wv (+ per-head ones column)
  S^T = K Q^T (contraction HD=64), expS = exp(S^T/8)
  x^T = V_aug^T expS (contraction Sk) -> divide by denominator row
  attn_out = (x^T)^T Wo + src + bo -> LN1 -> src1, src1T
  h^T = relu(W1^T src1T + bf1); ff = (h^T)^T W2 + bf2; out = LN2(src1 + ff)
"""

import numpy as np

D = 1024
S2 = 2048
SQ = 1024
PF = 4096
H = 16
HD = 64
DK = D // 128
PFK = PF // 128
NG = 4                 # head groups
HPG = H // NG
GW = HPG * HD          # 256 dims per group
GM = GW // 128
SCALE = 1.0 / 8.0
EPS = 1e-5
N_CORES = 8

_CACHE = {}


def _build():
    import concourse.bass as bass
    import concourse.mybir as mybir
    import concourse.tile as tile
    from concourse import bacc
    from concourse.masks import make_identity

    f32 = mybir.dt.float32
    bf16 = mybir.dt.bfloat16
    AF = mybir.ActivationFunctionType
    ALU = mybir.AluOpType

    nc = bacc.Bacc("TRN2", target_bir_lowering=False, debug=False, num_devices=N_CORES)

    def din(name, shape, dt=f32):
        return nc.dram_tensor(name, shape, dt, kind="ExternalInput")

    src_q = din("src_q", [SQ, D])
    src_o = din("src_o", [SQ, D])
    Wq = din("Wq", [D, D], bf16)
    Wk = din("Wk", [D, D], bf16)
    Wv = din("Wv", [D, D], bf16)
    Wo = din("Wo", [D, D], bf16)
    W1 = din("W1", [D, PF], bf16)
    W2 = din("W2", [PF, D], bf16)
    bq = din("bq", [D])
    bk = din("bk", [D])
    bv = din("bv", [D])
    bo = din("bo", [D])
    bf1 = din("bf1", [PF])
    bf2 = din("bf2", [D])
    g1 = din("g1", [D])
    b1 = din("b1", [D])
    g2 = din("g2", [D])
    b2 = din("b2", [D])
    out = nc.dram_tensor("out", [SQ, D], f32, kind="ExternalOutput")

    xT_dram = nc.dram_tensor("xT_scratch", [D, SQ], bf16)
    src1_dram = nc.dram_tensor("src1_scratch", [SQ, D], f32)

    def bc_ap(vec, n):
        return bass.AP(tensor=vec, offset=0, ap=[[0, 128], [1, n]])

    def col_ap(vec, m):
        return bass.AP(tensor=vec, offset=0, ap=[[1, 128], [128, m]])

    with tile.TileContext(nc) as tc:
        import contextlib

        with contextlib.ExitStack() as ctx:
            consts = ctx.enter_context(tc.tile_pool(name="consts", bufs=1))
            psum = ctx.enter_context(tc.tile_pool(name="psum", bufs=1, space="PSUM"))

            identity = consts.tile([128, 128], f32)
            make_identity(nc, identity)

            bq_col = consts.tile([128, DK], f32)
            nc.sync.dma_start(out=bq_col, in_=col_ap(bq, DK))
            bk_col = consts.tile([128, DK], f32)
            nc.sync.dma_start(out=bk_col, in_=col_ap(bk, DK))
            bf1_col = consts.tile([128, PFK], f32)
            nc.sync.dma_start(out=bf1_col, in_=col_ap(bf1, PFK))

            eps_t = consts.tile([128, 1], f32)
            nc.vector.memset(eps_t, EPS)

            def layer_norm(r_row, g_bc, b_bc, out_tile, tmp_pool):
                stats = tmp_pool.tile([128, 2, 6], f32, tag="ln_stats")
                rr = r_row.rearrange("p (a f) -> p a f", a=2)
                for a in range(2):
                    nc.vector.bn_stats(out=stats[:, a, :], in_=rr[:, a, :])
                mv = tmp_pool.tile([128, 2], f32, tag="ln_mv")
                nc.vector.bn_aggr(out=mv, in_=stats)
                rstd = tmp_pool.tile([128, 1], f32, tag="ln_rstd")
                nc.scalar.activation(
                    out=rstd, in_=mv[:, 1:2], func=AF.Sqrt, bias=eps_t, scale=1.0
                )
                nc.vector.reciprocal_approx_fast(out=rstd, in_=rstd)
                nc.vector.tensor_scalar(
                    out=out_tile,
                    in0=r_row,
                    scalar1=mv[:, 0:1],
                    scalar2=rstd,
                    op0=ALU.subtract,
                    op1=ALU.mult,
                )
                nc.vector.tensor_mul(out=out_tile, in0=out_tile, in1=g_bc)
                nc.vector.tensor_add(out=out_tile, in0=out_tile, in1=b_bc)

            # ============ Phase 0 + attention ============
            with contextlib.ExitStack() as attn_ctx:
                srctp = attn_ctx.enter_context(tc.tile_pool(name="srctp", bufs=1))
                srcT = srctp.tile([128, DK, S2], bf16)  # 4MB

                with tc.tile_pool(name="ph0", bufs=2) as ph0:
                    for blk in range(4):
                        half = src_q if blk < 2 else src_o
                        base = blk * 512
                        sts = []
                        for j in range(4):
                            st = ph0.tile([128, D], f32, tag="src_ld", bufs=6)
                            row0 = (blk % 2) * 512 + j * 128
                            nc.sync.dma_start(out=st, in_=half[row0 : row0 + 128, :])
                            sts.append(st)
                        for k in range(DK):
                            ps = psum.tile([128, 512], f32, tag="big", bufs=2)
                            for j in range(4):
                                nc.tensor.transpose(
                                    ps[:, j * 128 : (j + 1) * 128],
                                    sts[j][:, k * 128 : (k + 1) * 128],
                                    identity,
                                )
                            nc.vector.tensor_copy(
                                out=srcT[:, k, base : base + 512], in_=ps
                            )

                acts = attn_ctx.enter_context(tc.tile_pool(name="acts", bufs=1))
                bv_bc = acts.tile([128, D], f32)
                nc.gpsimd.dma_start(out=bv_bc, in_=bc_ap(bv, D))

                grp = attn_ctx.enter_context(tc.tile_pool(name="grp", bufs=2))
                wst = attn_ctx.enter_context(tc.tile_pool(name="wst", bufs=2))
                expp = attn_ctx.enter_context(tc.tile_pool(name="expp", bufs=2))
                nrm = attn_ctx.enter_context(tc.tile_pool(name="nrm", bufs=2))

                for g in range(NG):
                    gc0 = g * GW

                    wk_s = wst.tile([128, DK, GW], bf16, tag="wk_s")
                    nc.sync.dma_start(
                        out=wk_s,
                        in_=Wk.rearrange("(a p) n -> p a n", p=128)[:, :, gc0 : gc0 + GW],
                    )
                    wq_s = wst.tile([128, DK, GW], bf16, tag="wq_s")
                    nc.sync.dma_start(
                        out=wq_s,
                        in_=Wq.rearrange("(a p) n -> p a n", p=128)[:, :, gc0 : gc0 + GW],
                    )
                    wv_s = wst.tile([128, DK, GW], bf16, tag="wv_s")
                    nc.sync.dma_start(
                        out=wv_s,
                        in_=Wv.rearrange("(a p) n -> p a n", p=128)[:, :, gc0 : gc0 + GW],
                    )

                    # -- KT_g [GW, S2] bf16: 2 blocks of 1024, weight held per k --
                    KT = grp.tile([128, GM, S2], bf16, tag="KT")
                    for m in range(GM):
                        for nn in range(S2 // 1024):
                            ps = psum.tile([128, 1024], f32, tag="big", bufs=2)
                            for k in range(DK):
                                for hf in range(2):
                                    nc.tensor.matmul(
                                        ps[:, hf * 512 : (hf + 1) * 512],
                                        wk_s[:, k, m * 128 : (m + 1) * 128],
                                        srcT[
                                            :,
                                            k,
                                            nn * 1024 + hf * 512 : nn * 1024
                                            + (hf + 1) * 512,
                                        ],
                                        start=(k == 0),
                                        stop=(k == DK - 1),
                                    )
                            nc.vector.tensor_scalar_add(
                                out=KT[:, m, nn * 1024 : (nn + 1) * 1024],
                                in0=ps,
                                scalar1=bk_col[
                                    :, (gc0 // 128) + m : (gc0 // 128) + m + 1
                                ],
                            )

                    # -- QT_g [GW, SQ] bf16 --
                    QT = grp.tile([128, GM, SQ], bf16, tag="QT")
                    for m in range(GM):
                        ps = psum.tile([128, 1024], f32, tag="big", bufs=2)
                        for k in range(DK):
                            for hf in range(2):
                                nc.tensor.matmul(
                                    ps[:, hf * 512 : (hf + 1) * 512],
                                    wq_s[:, k, m * 128 : (m + 1) * 128],
                                    srcT[:, k, hf * 512 : (hf + 1) * 512],
                                    start=(k == 0),
                                    stop=(k == DK - 1),
                                )
                        nc.vector.tensor_scalar_add(
                            out=QT[:, m, :],
                            in0=ps,
                            scalar1=bq_col[:, (gc0 // 128) + m : (gc0 // 128) + m + 1],
                        )

                    # -- V_g [S2, HPG*(HD+1)] bf16 --
                    V = grp.tile([128, S2 // 128, HPG, HD + 1], bf16, tag="V")
                    nc.vector.memset(V[:, :, :, HD : HD + 1], 1.0)
                    for ms in range(S2 // 128):
                        ps = psum.tile([128, GW], f32, tag="big", bufs=2)
                        for k in range(DK):
                            nc.tensor.matmul(
                                ps,
                                srcT[:, k, ms * 128 : (ms + 1) * 128],
                                wv_s[:, k, :],
                                start=(k == 0),
                                stop=(k == DK - 1),
                            )
                        nc.vector.tensor_add(
                            out=V[:, ms, :, 0:HD],
                            in0=ps.rearrange("p (h d) -> p h d", h=HPG),
                            in1=bv_bc.rearrange("p (h d) -> p h d", h=H)[
                                :, HPG * g : HPG * (g + 1), :
                            ],
                        )

                    # -- attention: per head, both sq halves share each psum --
                    for hh in range(HPG):
                        m_h = hh // 2
                        p0 = (hh % 2) * 64
                        expS = expp.tile([128, S2 // 128, SQ], bf16, tag="expS")
                        for sk in range(S2 // 128):
                            ps = psum.tile([128, 1024], f32, tag="big", bufs=2)
                            for sq in range(2):
                                nc.tensor.matmul(
                                    ps[:, sq * 512 : (sq + 1) * 512],
                                    KT[p0 : p0 + 64, m_h, sk * 128 : (sk + 1) * 128],
                                    QT[p0 : p0 + 64, m_h, sq * 512 : (sq + 1) * 512],
                                    start=True,
                                    stop=True,
                                )
                            nc.scalar.activation(
                                out=expS[:, sk, :], in_=ps, func=AF.Exp, scale=SCALE
                            )
                        pv = psum.tile([HD + 1, SQ], f32, tag="pv", bufs=2)
                        for sk in range(S2 // 128):
                            for sq in range(2):
                                nc.tensor.matmul(
                                    pv[:, sq * 512 : (sq + 1) * 512],
                                    V[:, sk, hh, :],
                                    expS[:, sk, sq * 512 : (sq + 1) * 512],
                                    start=(sk == 0),
                                    stop=(sk == S2 // 128 - 1),
                                )
                        den = nrm.tile([1, SQ], f32, tag="den")
                        nc.vector.tensor_copy(out=den, in_=pv[HD : HD + 1, :])
                        den_bc = nrm.tile([64, SQ], f32, tag="den_bc")
                        nc.gpsimd.partition_broadcast(den_bc, den)
                        nc.vector.reciprocal_approx_fast(out=den_bc, in_=den_bc)
                        xt = nrm.tile([64, SQ], bf16, tag="xt")
                        nc.vector.tensor_mul(out=xt, in0=pv[0:HD, :], in1=den_bc)
                        h_abs = g * HPG + hh
                        nc.sync.dma_start(
                            out=xT_dram[h_abs * HD : (h_abs + 1) * HD, :], in_=xt
                        )

            # ============ out-projection + LN1 + FFN ============
            with contextlib.ExitStack() as fo_ctx:
                src1T_pool = fo_ctx.enter_context(tc.tile_pool(name="src1T", bufs=1))
                src1T = src1T_pool.tile([128, DK, SQ], bf16)

                with contextlib.ExitStack() as octx:
                    opool = octx.enter_context(tc.tile_pool(name="oproj", bufs=1))
                    otmp = octx.enter_context(tc.tile_pool(name="otmp", bufs=2))

                    bo_bc = opool.tile([128, D], f32)
                    nc.gpsimd.dma_start(out=bo_bc, in_=bc_ap(bo, D))
                    g1_bc = opool.tile([128, D], f32)
                    nc.gpsimd.dma_start(out=g1_bc, in_=bc_ap(g1, D))
                    b1_bc = opool.tile([128, D], f32)
                    nc.gpsimd.dma_start(out=b1_bc, in_=bc_ap(b1, D))

                    wo_s = opool.tile([128, DK, D], bf16)
                    xts = opool.tile([128, DK, SQ], bf16)
                    for k in range(DK):
                        nc.sync.dma_start(
                            out=xts[:, k, :], in_=xT_dram[k * 128 : (k + 1) * 128, :]
                        )
                        nc.sync.dma_start(
                            out=wo_s[:, k, :], in_=Wo[k * 128 : (k + 1) * 128, :]
                        )

                    r_sb = opool.tile([128, SQ // 128, D], f32)

                    for m in range(SQ // 128):
                        ps = psum.tile([128, 1024], f32, tag="big", bufs=2)
                        for k in range(DK):
                            for n in range(2):
                                nc.tensor.matmul(
                                    ps[:, n * 512 : (n + 1) * 512],
                                    xts[:, k, m * 128 : (m + 1) * 128],
                                    wo_s[:, k, n * 512 : (n + 1) * 512],
                                    start=(k == 0),
                                    stop=(k == DK - 1),
                                )
                        sq_t = otmp.tile([128, D], f32, tag="sq_ld", bufs=3)
                        nc.sync.dma_start(out=sq_t, in_=src_q[m * 128 : (m + 1) * 128, :])
                        nc.vector.tensor_add(out=r_sb[:, m, :], in0=ps, in1=sq_t)

                    for m in range(SQ // 128):
                        rrow = r_sb[:, m, :]
                        nc.vector.tensor_add(out=rrow, in0=rrow, in1=bo_bc)
                        s1 = otmp.tile([128, D], f32, tag="s1", bufs=3)
                        layer_norm(rrow, g1_bc, b1_bc, s1, otmp)
                        nc.sync.dma_start(
                            out=src1_dram[m * 128 : (m + 1) * 128, :], in_=s1
                        )
                        for kk in range(2):
                            ps = psum.tile([128, 512], f32, tag="big", bufs=2)
                            for j in range(4):
                                k = kk * 4 + j
                                nc.tensor.transpose(
                                    ps[:, j * 128 : (j + 1) * 128],
                                    s1[:, k * 128 : (k + 1) * 128],
                                    identity,
                                )
                            for j in range(4):
                                k = kk * 4 + j
                                nc.vector.tensor_copy(
                                    out=src1T[:, k, m * 128 : (m + 1) * 128],
                                    in_=ps[:, j * 128 : (j + 1) * 128],
                                )

                # ============ FFN ============
                with contextlib.ExitStack() as fctx:
                    hpool = fctx.enter_context(tc.tile_pool(name="hpool", bufs=1))
                    hT = hpool.tile([128, PFK, SQ], bf16)     # 8MB
                    w2p = fctx.enter_context(tc.tile_pool(name="w2p", bufs=1))
                    w2bf = w2p.tile([128, PFK, D], bf16)      # 8MB
                    fcts = fctx.enter_context(tc.tile_pool(name="fcts", bufs=1))
                    ftmp = fctx.enter_context(tc.tile_pool(name="ftmp", bufs=2))

                    bf2_bc = fcts.tile([128, D], f32)
                    nc.gpsimd.dma_start(out=bf2_bc, in_=bc_ap(bf2, D))
                    g2_bc = fcts.tile([128, D], f32)
                    nc.gpsimd.dma_start(out=g2_bc, in_=bc_ap(g2, D))
                    b2_bc = fcts.tile([128, D], f32)
                    nc.gpsimd.dma_start(out=b2_bc, in_=bc_ap(b2, D))

                    nc.sync.dma_start(
                        out=w2bf, in_=W2.rearrange("(a p) n -> p a n", p=128)
                    )

                    # FFN1: hT[pf, q] = relu(W1^T src1T + bf1), sq-half passes
                    for sqh in range(2):
                        for mp in range(PFK):
                            w1_s = ftmp.tile([128, DK, 128], bf16, tag="w1_s", bufs=3)
                            nc.sync.dma_start(
                                out=w1_s,
                                in_=W1.rearrange("(a p) n -> p a n", p=128)[
                                    :, :, mp * 128 : (mp + 1) * 128
                                ],
                            )
                            ps = psum.tile([128, 512], f32, tag="big", bufs=2)
                            for k in range(DK):
                                nc.tensor.matmul(
                                    ps,
                                    w1_s[:, k, :],
                                    src1T[:, k, sqh * 512 : (sqh + 1) * 512],
                                    start=(k == 0),
                                    stop=(k == DK - 1),
                                )
                            nc.vector.tensor_scalar(
                                out=hT[:, mp, sqh * 512 : (sqh + 1) * 512],
                                in0=ps,
                                scalar1=bf1_col[:, mp : mp + 1],
                                scalar2=0.0,
                                op0=ALU.add,
                                op1=ALU.max,
                            )

                    # FFN2 per m row + residual + LN2
                    for m in range(SQ // 128):
                        ps = psum.tile([128, 1024], f32, tag="big", bufs=2)
                        for k in range(PFK):
                            for n in range(2):
                                nc.tensor.matmul(
                                    ps[:, n * 512 : (n + 1) * 512],
                                    hT[:, k, m * 128 : (m + 1) * 128],
                                    w2bf[:, k, n * 512 : (n + 1) * 512],
                                    start=(k == 0),
                                    stop=(k == PFK - 1),
                                )
                        s1_t = ftmp.tile([128, D], f32, tag="s1_ld", bufs=2)
                        nc.sync.dma_start(
                            out=s1_t, in_=src1_dram[m * 128 : (m + 1) * 128, :]
                        )
                        rr = ftmp.tile([128, D], f32, tag="rr", bufs=2)
                        nc.vector.tensor_add(out=rr, in0=ps, in1=s1_t)
                        nc.vector.tensor_add(out=rr, in0=rr, in1=bf2_bc)
                        layer_norm(rr, g2_bc, b2_bc, rr, ftmp)
                        nc.sync.dma_start(out=out[m * 128 : (m + 1) * 128, :], in_=rr)

    nc.compile()
    return nc


def kernel(**inputs):
    import ml_dtypes
    from concourse.bass_utils import run_bass_kernel_spmd

    if "nc" not in _CACHE:
        _CACHE["nc"] = _build()
    nc = _CACHE["nc"]

    ins = {k: np.asarray(v, dtype=np.float32) for k, v in inputs.items()}
    src = ins["src"]
    bf = ml_dtypes.bfloat16
    weights = {}
    for n in ["Wq", "Wk", "Wv", "Wo", "W1", "W2"]:
        weights[n] = np.ascontiguousarray(ins[n]).astype(bf)
    for n in ["bq", "bk", "bv", "bo", "bf1", "bf2", "g1", "b1", "g2", "b2"]:
        weights[n] = np.ascontiguousarray(ins[n])

    in_maps = []
    for c in range(N_CORES):
        b, h = divmod(c, 2)
        m = dict(weights)
        m["src_q"] = np.ascontiguousarray(src[b, h * SQ : (h + 1) * SQ])
        m["src_o"] = np.ascontiguousarray(src[b, (1 - h) * SQ : (2 - h) * SQ])
        in_maps.append(m)

    res = run_bass_kernel_spmd(nc, in_maps, list(range(N_CORES)))

    out = np.empty((4, S2, D), dtype=np.float32)
    for c in range(N_CORES):
        b, h = divmod(c, 2)
        out[b, h * SQ : (h + 1) * SQ] = res.results[c]["out"]
    return out


# revision 18
# speedup vs baseline: 12290.2209x; 12290.2209x over previous
"""Trainium2 Bass kernel for a transformer encoder layer.

B=4, S=2048, D=1024, H=16 heads (HD=64), PF=4096, fp32 I/O.

Sharding: 8 cores, core c handles batch c//2, query seq-half c%2 (1024
tokens). Each core computes K/V over its batch's full 2048-token sequence
(duplicated within the pair; ~12% extra flops) so no collectives are needed.

All matmuls run in bf16 (weights cast host-side; activations cast at PSUM
eviction), accumulating in fp32 PSUM. Residual/LayerNorm arithmetic stays
fp32. Attention is kept transposed so no P-matrix transpose is needed:
  srcT = src^T; QT/KT = W^T srcT; V = srcT^T Wv (+ per-head ones column)
  S^T = K Q^T (contraction HD=64), expS = exp(S^T/8)
  x^T = V_aug^T expS (contraction Sk) -> divide by denominator row
  attn_out = (x^T)^T Wo + src + bo -> LN1 -> src1, src1T
  h^T = relu(W1^T src1T + bf1); ff = (h^T)^T W2 + bf2; out = LN2(src1 + ff)
"""

import numpy as np

D = 1024
S2 = 2048
SQ = 1024
PF = 4096
H = 16
HD = 64
DK = D // 128
PFK = PF // 128
NG = 4                 # head groups
HPG = H // NG
GW = HPG * HD          # 256 dims per group
GM = GW // 128
SCALE = 1.0 / 8.0
EPS = 1e-5
N_CORES = 8

_CACHE = {}


def _build():
    import concourse.bass as bass
    import concourse.mybir as mybir
    import concourse.tile as tile
    from concourse import bacc
    from concourse.masks import make_identity

    f32 = mybir.dt.float32
    bf16 = mybir.dt.bfloat16
    AF = mybir.ActivationFunctionType
    ALU = mybir.AluOpType

    nc = bacc.Bacc("TRN2", target_bir_lowering=False, debug=False, num_devices=N_CORES)

    def din(name, shape, dt=f32):
        return nc.dram_tensor(name, shape, dt, kind="ExternalInput")

    src_q = din("src_q", [SQ, D])
    src_o = din("src_o", [SQ, D])
    Wq = din("Wq", [D, D], bf16)
    Wk = din("Wk", [D, D], bf16)
    Wv = din("Wv", [D, D], bf16)
    Wo = din("Wo", [D, D], bf16)
    W1 = din("W1", [D, PF], bf16)
    W2 = din("W2", [PF, D], bf16)
    bq = din("bq", [D])
    bk = din("bk", [D])
    bv = din("bv", [D])
    bo = din("bo", [D])
    bf1 = din("bf1", [PF])
    bf2 = din("bf2", [D])
    g1 = din("g1", [D])
    b1 = din("b1", [D])
    g2 = din("g2", [D])
    b2 = din("b2", [D])
    out = nc.dram_tensor("out", [SQ, D], f32, kind="ExternalOutput")

    xT_dram = nc.dram_tensor("xT_scratch", [D, SQ], bf16)
    src1_dram = nc.dram_tensor("src1_scratch", [SQ, D], f32)

    def bc_ap(vec, n):
        return bass.AP(tensor=vec, offset=0, ap=[[0, 128], [1, n]])

    def col_ap(vec, m):
        return bass.AP(tensor=vec, offset=0, ap=[[1, 128], [128, m]])

    with tile.TileContext(nc) as tc:
        import contextlib

        with contextlib.ExitStack() as ctx:
            consts = ctx.enter_context(tc.tile_pool(name="consts", bufs=1))
            psum = ctx.enter_context(tc.tile_pool(name="psum", bufs=1, space="PSUM"))

            identity = consts.tile([128, 128], f32)
            make_identity(nc, identity)

            bq_col = consts.tile([128, DK], f32)
            nc.sync.dma_start(out=bq_col, in_=col_ap(bq, DK))
            bk_col = consts.tile([128, DK], f32)
            nc.sync.dma_start(out=bk_col, in_=col_ap(bk, DK))
            bf1_col = consts.tile([128, PFK], f32)
            nc.sync.dma_start(out=bf1_col, in_=col_ap(bf1, PFK))

            eps_t = consts.tile([128, 1], f32)
            nc.vector.memset(eps_t, EPS)

            def layer_norm(r_row, g_bc, b_bc, out_tile, tmp_pool):
                stats = tmp_pool.tile([128, 2, 6], f32, tag="ln_stats")
                rr = r_row.rearrange("p (a f) -> p a f", a=2)
                for a in range(2):
                    nc.vector.bn_stats(out=stats[:, a, :], in_=rr[:, a, :])
                mv = tmp_pool.tile([128, 2], f32, tag="ln_mv")
                nc.vector.bn_aggr(out=mv, in_=stats)
                rstd = tmp_pool.tile([128, 1], f32, tag="ln_rstd")
                nc.scalar.activation(
                    out=rstd, in_=mv[:, 1:2], func=AF.Sqrt, bias=eps_t, scale=1.0
                )
                nc.vector.reciprocal_approx_fast(out=rstd, in_=rstd)
                nc.vector.tensor_scalar(
                    out=out_tile,
                    in0=r_row,
                    scalar1=mv[:, 0:1],
                    scalar2=rstd,
                    op0=ALU.subtract,
                    op1=ALU.mult,
                )
                nc.gpsimd.tensor_mul(out=out_tile, in0=out_tile, in1=g_bc)
                nc.gpsimd.tensor_add(out=out_tile, in0=out_tile, in1=b_bc)

            # ============ Phase 0 + attention ============
            with contextlib.ExitStack() as attn_ctx:
                srctp = attn_ctx.enter_context(tc.tile_pool(name="srctp", bufs=1))
                srcT = srctp.tile([128, DK, S2], bf16)  # 4MB

                with tc.tile_pool(name="ph0", bufs=2) as ph0:
                    for blk in range(4):
                        half = src_q if blk < 2 else src_o
                        base = blk * 512
                        sts = []
                        for j in range(4):
                            st = ph0.tile([128, D], f32, tag="src_ld", bufs=6)
                            row0 = (blk % 2) * 512 + j * 128
                            nc.sync.dma_start(out=st, in_=half[row0 : row0 + 128, :])
                            sts.append(st)
                        for k in range(DK):
                            ps = psum.tile([128, 512], f32, tag="big", bufs=2)
                            for j in range(4):
                                nc.tensor.transpose(
                                    ps[:, j * 128 : (j + 1) * 128],
                                    sts[j][:, k * 128 : (k + 1) * 128],
                                    identity,
                                )
                            nc.vector.tensor_copy(
                                out=srcT[:, k, base : base + 512], in_=ps
                            )

                acts = attn_ctx.enter_context(tc.tile_pool(name="acts", bufs=1))
                bv_bc = acts.tile([128, D], f32)
                nc.gpsimd.dma_start(out=bv_bc, in_=bc_ap(bv, D))

                grp = attn_ctx.enter_context(tc.tile_pool(name="grp", bufs=2))
                wst = attn_ctx.enter_context(tc.tile_pool(name="wst", bufs=2))
                expp = attn_ctx.enter_context(tc.tile_pool(name="expp", bufs=2))
                nrm = attn_ctx.enter_context(tc.tile_pool(name="nrm", bufs=2))

                for g in range(NG):
                    gc0 = g * GW

                    wk_s = wst.tile([128, DK, GW], bf16, tag="wk_s")
                    nc.sync.dma_start(
                        out=wk_s,
                        in_=Wk.rearrange("(a p) n -> p a n", p=128)[:, :, gc0 : gc0 + GW],
                    )
                    wq_s = wst.tile([128, DK, GW], bf16, tag="wq_s")
                    nc.sync.dma_start(
                        out=wq_s,
                        in_=Wq.rearrange("(a p) n -> p a n", p=128)[:, :, gc0 : gc0 + GW],
                    )
                    wv_s = wst.tile([128, DK, GW], bf16, tag="wv_s")
                    nc.sync.dma_start(
                        out=wv_s,
                        in_=Wv.rearrange("(a p) n -> p a n", p=128)[:, :, gc0 : gc0 + GW],
                    )

                    # -- KT_g [GW, S2] bf16: 2 blocks of 1024, weight held per k --
                    KT = grp.tile([128, GM, S2], bf16, tag="KT")
                    for m in range(GM):
                        for nn in range(S2 // 1024):
                            ps = psum.tile([128, 1024], f32, tag="big", bufs=2)
                            for k in range(DK):
                                for hf in range(2):
                                    nc.tensor.matmul(
                                        ps[:, hf * 512 : (hf + 1) * 512],
                                        wk_s[:, k, m * 128 : (m + 1) * 128],
                                        srcT[
                                            :,
                                            k,
                                            nn * 1024 + hf * 512 : nn * 1024
                                            + (hf + 1) * 512,
                                        ],
                                        start=(k == 0),
                                        stop=(k == DK - 1),
                                    )
                            nc.vector.tensor_scalar_add(
                                out=KT[:, m, nn * 1024 : (nn + 1) * 1024],
                                in0=ps,
                                scalar1=bk_col[
                                    :, (gc0 // 128) + m : (gc0 // 128) + m + 1
                                ],
                            )

                    # -- QT_g [GW, SQ] bf16 --
                    QT = grp.tile([128, GM, SQ], bf16, tag="QT")
                    for m in range(GM):
                        ps = psum.tile([128, 1024], f32, tag="big", bufs=2)
                        for k in range(DK):
                            for hf in range(2):
                                nc.tensor.matmul(
                                    ps[:, hf * 512 : (hf + 1) * 512],
                                    wq_s[:, k, m * 128 : (m + 1) * 128],
                                    srcT[:, k, hf * 512 : (hf + 1) * 512],
                                    start=(k == 0),
                                    stop=(k == DK - 1),
                                )
                        nc.vector.tensor_scalar_add(
                            out=QT[:, m, :],
                            in0=ps,
                            scalar1=bq_col[:, (gc0 // 128) + m : (gc0 // 128) + m + 1],
                        )

                    # -- V_g [S2, HPG*(HD+1)] bf16 --
                    V = grp.tile([128, S2 // 128, HPG, HD + 1], bf16, tag="V")
                    nc.vector.memset(V[:, :, :, HD : HD + 1], 1.0)
                    for ms in range(S2 // 128):
                        ps = psum.tile([128, GW], f32, tag="big", bufs=2)
                        for k in range(DK):
                            nc.tensor.matmul(
                                ps,
                                srcT[:, k, ms * 128 : (ms + 1) * 128],
                                wv_s[:, k, :],
                                start=(k == 0),
                                stop=(k == DK - 1),
                            )
                        nc.vector.tensor_add(
                            out=V[:, ms, :, 0:HD],
                            in0=ps.rearrange("p (h d) -> p h d", h=HPG),
                            in1=bv_bc.rearrange("p (h d) -> p h d", h=H)[
                                :, HPG * g : HPG * (g + 1), :
                            ],
                        )

                    # -- attention: per head, both sq halves share each psum --
                    for hh in range(HPG):
                        m_h = hh // 2
                        p0 = (hh % 2) * 64
                        expS = expp.tile([128, S2 // 128, SQ], bf16, tag="expS")
                        for sk in range(S2 // 128):
                            ps = psum.tile([128, 1024], f32, tag="big", bufs=2)
                            for sq in range(2):
                                nc.tensor.matmul(
                                    ps[:, sq * 512 : (sq + 1) * 512],
                                    KT[p0 : p0 + 64, m_h, sk * 128 : (sk + 1) * 128],
                                    QT[p0 : p0 + 64, m_h, sq * 512 : (sq + 1) * 512],
                                    start=True,
                                    stop=True,
                                )
                            nc.scalar.activation(
                                out=expS[:, sk, :], in_=ps, func=AF.Exp, scale=SCALE
                            )
                        pv = psum.tile([HD + 1, SQ], f32, tag="pv", bufs=2)
                        for sk in range(S2 // 128):
                            for sq in range(2):
                                nc.tensor.matmul(
                                    pv[:, sq * 512 : (sq + 1) * 512],
                                    V[:, sk, hh, :],
                                    expS[:, sk, sq * 512 : (sq + 1) * 512],
                                    start=(sk == 0),
                                    stop=(sk == S2 // 128 - 1),
                                )
                        den = nrm.tile([1, SQ], f32, tag="den")
                        nc.vector.tensor_copy(out=den, in_=pv[HD : HD + 1, :])
                        den_bc = nrm.tile([64, SQ], f32, tag="den_bc")
                        nc.gpsimd.partition_broadcast(den_bc, den)
                        nc.vector.reciprocal_approx_fast(out=den_bc, in_=den_bc)
                        xt = nrm.tile([64, SQ], bf16, tag="xt")
                        nc.vector.tensor_mul(out=xt, in0=pv[0:HD, :], in1=den_bc)
                        h_abs = g * HPG + hh
                        nc.sync.dma_start(
                            out=xT_dram[h_abs * HD : (h_abs + 1) * HD, :], in_=xt
                        )

            # ============ out-projection + LN1 + FFN ============
            with contextlib.ExitStack() as fo_ctx:
                src1T_pool = fo_ctx.enter_context(tc.tile_pool(name="src1T", bufs=1))
                src1T = src1T_pool.tile([128, DK, SQ], bf16)

                with contextlib.ExitStack() as octx:
                    opool = octx.enter_context(tc.tile_pool(name="oproj", bufs=1))
                    otmp = octx.enter_context(tc.tile_pool(name="otmp", bufs=2))

                    bo_bc = opool.tile([128, D], f32)
                    nc.gpsimd.dma_start(out=bo_bc, in_=bc_ap(bo, D))
                    g1_bc = opool.tile([128, D], f32)
                    nc.gpsimd.dma_start(out=g1_bc, in_=bc_ap(g1, D))
                    b1_bc = opool.tile([128, D], f32)
                    nc.gpsimd.dma_start(out=b1_bc, in_=bc_ap(b1, D))

                    wo_s = opool.tile([128, DK, D], bf16)
                    xts = opool.tile([128, DK, SQ], bf16)
                    for k in range(DK):
                        nc.sync.dma_start(
                            out=xts[:, k, :], in_=xT_dram[k * 128 : (k + 1) * 128, :]
                        )
                        nc.sync.dma_start(
                            out=wo_s[:, k, :], in_=Wo[k * 128 : (k + 1) * 128, :]
                        )

                    r_sb = opool.tile([128, SQ // 128, D], f32)

                    for m in range(SQ // 128):
                        ps = psum.tile([128, 1024], f32, tag="big", bufs=2)
                        for k in range(DK):
                            for n in range(2):
                                nc.tensor.matmul(
                                    ps[:, n * 512 : (n + 1) * 512],
                                    xts[:, k, m * 128 : (m + 1) * 128],
                                    wo_s[:, k, n * 512 : (n + 1) * 512],
                                    start=(k == 0),
                                    stop=(k == DK - 1),
                                )
                        sq_t = otmp.tile([128, D], f32, tag="sq_ld", bufs=3)
                        nc.sync.dma_start(out=sq_t, in_=src_q[m * 128 : (m + 1) * 128, :])
                        nc.vector.tensor_add(out=r_sb[:, m, :], in0=ps, in1=sq_t)

                    for m in range(SQ // 128):
                        rrow = r_sb[:, m, :]
                        nc.gpsimd.tensor_add(out=rrow, in0=rrow, in1=bo_bc)
                        s1 = otmp.tile([128, D], f32, tag="s1", bufs=3)
                        layer_norm(rrow, g1_bc, b1_bc, s1, otmp)
                        nc.sync.dma_start(
                            out=src1_dram[m * 128 : (m + 1) * 128, :], in_=s1
                        )
                        for kk in range(2):
                            ps = psum.tile([128, 512], f32, tag="big", bufs=2)
                            for j in range(4):
                                k = kk * 4 + j
                                nc.tensor.transpose(
                                    ps[:, j * 128 : (j + 1) * 128],
                                    s1[:, k * 128 : (k + 1) * 128],
                                    identity,
                                )
                            for j in range(4):
                                k = kk * 4 + j
                                nc.vector.tensor_copy(
                                    out=src1T[:, k, m * 128 : (m + 1) * 128],
                                    in_=ps[:, j * 128 : (j + 1) * 128],
                                )

                # ============ FFN ============
                with contextlib.ExitStack() as fctx:
                    hpool = fctx.enter_context(tc.tile_pool(name="hpool", bufs=1))
                    hT = hpool.tile([128, PFK, SQ], bf16)     # 8MB
                    w2p = fctx.enter_context(tc.tile_pool(name="w2p", bufs=1))
                    w2bf = w2p.tile([128, PFK, D], bf16)      # 8MB
                    fcts = fctx.enter_context(tc.tile_pool(name="fcts", bufs=1))
                    ftmp = fctx.enter_context(tc.tile_pool(name="ftmp", bufs=2))

                    bf2_bc = fcts.tile([128, D], f32)
                    nc.gpsimd.dma_start(out=bf2_bc, in_=bc_ap(bf2, D))
                    g2_bc = fcts.tile([128, D], f32)
                    nc.gpsimd.dma_start(out=g2_bc, in_=bc_ap(g2, D))
                    b2_bc = fcts.tile([128, D], f32)
                    nc.gpsimd.dma_start(out=b2_bc, in_=bc_ap(b2, D))

                    nc.sync.dma_start(
                        out=w2bf, in_=W2.rearrange("(a p) n -> p a n", p=128)
                    )

                    # FFN1: hT[pf, q] = relu(W1^T src1T + bf1), sq-half passes
                    for sqh in range(2):
                        for mp in range(PFK):
                            w1_s = ftmp.tile([128, DK, 128], bf16, tag="w1_s", bufs=3)
                            nc.sync.dma_start(
                                out=w1_s,
                                in_=W1.rearrange("(a p) n -> p a n", p=128)[
                                    :, :, mp * 128 : (mp + 1) * 128
                                ],
                            )
                            ps = psum.tile([128, 512], f32, tag="big", bufs=2)
                            for k in range(DK):
                                nc.tensor.matmul(
                                    ps,
                                    w1_s[:, k, :],
                                    src1T[:, k, sqh * 512 : (sqh + 1) * 512],
                                    start=(k == 0),
                                    stop=(k == DK - 1),
                                )
                            nc.vector.tensor_scalar(
                                out=hT[:, mp, sqh * 512 : (sqh + 1) * 512],
                                in0=ps,
                                scalar1=bf1_col[:, mp : mp + 1],
                                scalar2=0.0,
                                op0=ALU.add,
                                op1=ALU.max,
                            )

                    # FFN2 per m row + residual + LN2
                    for m in range(SQ // 128):
                        ps = psum.tile([128, 1024], f32, tag="big", bufs=2)
                        for k in range(PFK):
                            for n in range(2):
                                nc.tensor.matmul(
                                    ps[:, n * 512 : (n + 1) * 512],
                                    hT[:, k, m * 128 : (m + 1) * 128],
                                    w2bf[:, k, n * 512 : (n + 1) * 512],
                                    start=(k == 0),
                                    stop=(k == PFK - 1),
                                )
                        s1_t = ftmp.tile([128, D], f32, tag="s1_ld", bufs=2)
                        nc.sync.dma_start(
                            out=s1_t, in_=src1_dram[m * 128 : (m + 1) * 128, :]
                        )
                        rr = ftmp.tile([128, D], f32, tag="rr", bufs=2)
                        nc.vector.tensor_add(out=rr, in0=ps, in1=s1_t)
                        nc.gpsimd.tensor_add(out=rr, in0=rr, in1=bf2_bc)
                        layer_norm(rr, g2_bc, b2_bc, rr, ftmp)
                        nc.sync.dma_start(out=out[m * 128 : (m + 1) * 128, :], in_=rr)

    nc.compile()
    return nc


def kernel(**inputs):
    import ml_dtypes
    from concourse.bass_utils import run_bass_kernel_spmd

    if "nc" not in _CACHE:
        _CACHE["nc"] = _build()
    nc = _CACHE["nc"]

    ins = {k: np.asarray(v, dtype=np.float32) for k, v in inputs.items()}
    src = ins["src"]
    bf = ml_dtypes.bfloat16
    weights = {}
    for n in ["Wq", "Wk", "Wv", "Wo", "W1", "W2"]:
        weights[n] = np.ascontiguousarray(ins[n]).astype(bf)
    for n in ["bq", "bk", "bv", "bo", "bf1", "bf2", "g1", "b1", "g2", "b2"]:
        weights[n] = np.ascontiguousarray(ins[n])

    in_maps = []
    for c in range(N_CORES):
        b, h = divmod(c, 2)
        m = dict(weights)
        m["src_q"] = np.ascontiguousarray(src[b, h * SQ : (h + 1) * SQ])
        m["src_o"] = np.ascontiguousarray(src[b, (1 - h) * SQ : (2 - h) * SQ])
        in_maps.append(m)

    res = run_bass_kernel_spmd(nc, in_maps, list(range(N_CORES)))

    out = np.empty((4, S2, D), dtype=np.float32)
    for c in range(N_CORES):
        b, h = divmod(c, 2)
        out[b, h * SQ : (h + 1) * SQ] = res.results[c]["out"]
    return out
